# revision 47
# baseline (speedup 1.0000x reference)
"""DRC-GCN v2: fp8 gather + DoubleRow staircase on 8 trn2 NeuronCores.

Strategy (node/data parallel, dest-sharded):
  - Destination nodes sharded across 8 cores (12500 -> padded 12544 rows).
  - M_l (the spmm operand) is replicated per core in DRAM as fp8(e4m3),
    value-scaled by 1/s_l (s_l = prod of |gamma|) so it stays in fp8 range.
  - Per layer: gather source rows per edge (256B fp8 descriptors) into
    per-block-pair G tiles; segment-sum via DoubleRow fp8 staircase matmuls
    (lhsT = segw[slot, dest] = 16*w one-hot, built batched on DVE).
  - acc is never materialized: logits accumulate directly through
    host-precomputed Vl = s_l * ((1-b)I + b W_l) @ W_sort  [256, 64].
  - Xc carry kept in SBUF bf16 at 1/s_l scale; Xn cast to fp8 and
    AllGather'd into M_{l+1}.
"""

import math

import numpy as np
import ml_dtypes

import concourse.bass as bass
import concourse.mybir as mybir
import concourse.tile as tile
from concourse import bacc
from concourse import bass_utils

FP32 = mybir.dt.float32
BF16 = mybir.dt.bfloat16
F8 = mybir.dt.float8e4
I16 = mybir.dt.int16
I32 = mybir.dt.int32
AF = mybir.ActivationFunctionType
ALU = mybir.AluOpType
PM = mybir.MatmulPerfMode

P = 128
CORES = 8
TAU = 0.5
BUCKET = 20096          # gather bucket rows (int16-addressable)
NI_MAX = 896            # max idxs per dma_gather call
DMA_SCRATCH = 16384     # SWDGE descriptor ring carveout (bytes/partition)
NQ = 4                  # SWDGE queues
KB = 16                 # segw build batch (chunks per DVE op)


def _round_up(x, m):
    return (x + m - 1) // m * m


class Plan:
    """Static kernel structure + per-core data arrays (block-pair layout)."""

    def __init__(self, n_nodes, nfeat, nhid, ncls, nlayers,
                 edge_row, edge_col, edge_w):
        self.n = n_nodes
        self.nfeat = nfeat
        self.nhid = nhid
        self.ncls = ncls
        self.nl = nlayers
        assert n_nodes % CORES == 0
        self.nsh_raw = n_nodes // CORES
        self.nsh = _round_up(self.nsh_raw, P)
        self.nb = self.nsh // P
        assert self.nb % 2 == 0
        self.npair = self.nb // 2
        self.ntot = self.nsh * CORES
        self.nbuck = (self.ntot + BUCKET - 1) // BUCKET

        own = edge_col // self.nsh_raw
        gsrc = own * self.nsh + (edge_col - own * self.nsh_raw)
        dst_core = edge_row // self.nsh_raw
        ldst = edge_row - dst_core * self.nsh_raw
        blk = ldst // P
        pair = blk // 2
        half = blk % 2
        ld_in_blk = ldst % P
        bucket = gsrc // BUCKET
        lidx = gsrc - bucket * BUCKET

        npair, nbuck = self.npair, self.nbuck
        counts = np.zeros((CORES, npair, nbuck, 2), np.int64)
        np.add.at(counts, (dst_core, pair, bucket, half), 1)
        caps = _round_up(counts.max(axis=0), P)            # [npair, nbuck, 2]
        self.caps = caps

        # chunk layout per pair: bucket-major, [A chunks | B chunks]
        # calls per (pair, bucket): one if niA+niB <= NI_MAX else two
        self.pcalls = []      # per pair: list of (bucket, chunk_off, ni, o16)
        self.mm = []          # per pair: [half] -> list of (c0, n) chunk runs
        self.chp = np.zeros(npair, np.int64)
        o16 = 0

        def _pieces(cap):
            # split cap into balanced multiples of 128, each <= NI_MAX
            out = []
            left = cap
            while left > 0:
                ni = min(left, NI_MAX)
                if left > NI_MAX:
                    ni = min(_round_up(left // 2, P), NI_MAX)
                out.append(ni)
                left -= ni
            return out

        # calls split per (pair, bucket, half) so per-core padding is always
        # trailing within a call; gcnt holds per-core real counts (>=16,
        # 16-aligned, trailing idxs set to -1) so the gather ucode skips
        # padding descriptors.
        self.cmeta = {}       # (pair, bucket, half) -> [(call_gidx, off, ni)]
        gidx = 0
        for p in range(npair):
            calls = []
            runs = [[], []]
            co = 0
            for k in range(nbuck):
                capA = int(caps[p, k, 0])
                capB = int(caps[p, k, 1])
                if capA + capB == 0:
                    continue
                cco = co
                for h, cap in ((0, capA), (1, capB)):
                    off = 0
                    metas = []
                    for ni in _pieces(cap):
                        calls.append((k, cco, ni, o16))
                        metas.append((gidx, off, ni))
                        gidx += 1
                        o16 += ni // 16
                        cco += ni // P
                        off += ni
                    if metas:
                        self.cmeta[(p, k, h)] = metas
                if capA:
                    runs[0].append((co, capA // P))
                if capB:
                    runs[1].append((co + capA // P, capB // P))
                co += (capA + capB) // P
            self.pcalls.append(calls)
            self.mm.append(runs)
            self.chp[p] = co
        self.ocp = np.concatenate([[0], np.cumsum(self.chp)]).astype(np.int64)
        self.totch = int(self.ocp[-1])
        self.chmax = int(self.chp.max())
        self.tot16 = o16
        self.ncalls = gidx
        self.pbase = np.concatenate(
            [[0], np.cumsum([len(c) for c in self.pcalls])]).astype(np.int64)
        self.gcnt = np.zeros((CORES, self.ncalls), np.int32)

        # ---- per-core data arrays ----
        order = np.lexsort((lidx, half, bucket, pair, dst_core))
        self.eidx = np.zeros((CORES, P, self.tot16), np.int16)
        self.segw = np.zeros((CORES, P, self.totch, P), ml_dtypes.float8_e4m3fn)

        s_core = dst_core[order]
        s_pair = pair[order]
        s_buck = bucket[order]
        s_half = half[order]
        s_lidx = lidx[order]
        s_ld = ld_in_blk[order]
        s_w = edge_w[order]

        core_starts = np.searchsorted(s_core, np.arange(CORES + 1))
        nseg = npair * nbuck * 2
        for c in range(CORES):
            a0, a1 = core_starts[c], core_starts[c + 1]
            key = (s_pair[a0:a1] * nbuck + s_buck[a0:a1]) * 2 + s_half[a0:a1]
            seg_starts = np.searchsorted(key, np.arange(nseg + 1)) + a0
            for p in range(npair):
                co = 0
                o16p = self.pcalls[p][0][3] if self.pcalls[p] else 0
                for k in range(self.nbuck):
                    for h in range(2):
                        cap = int(caps[p, k, h])
                        if cap == 0:
                            continue
                        s0 = seg_starts[(p * nbuck + k) * 2 + h]
                        s1 = seg_starts[(p * nbuck + k) * 2 + h + 1]
                        nreal = s1 - s0
                        idxs = np.full(cap, -1, np.int16)
                        idxs[:nreal] = s_lidx[s0:s1]
                        for (gi, off, ni) in self.cmeta.get((p, k, h), []):
                            real = min(max(nreal - off, 0), ni)
                            eff = min(ni, _round_up(max(real, 16), 16))
                            idxs[off + real:off + eff] = 0
                            self.gcnt[c, gi] = eff
                        jr = np.arange(nreal)
                        gco = self.ocp[p] + co
                        self.segw[c, jr % P, gco + jr // P,
                                  s_ld[s0:s1]] = np.clip(
                            16.0 * s_w[s0:s1], 0, 240).astype(
                            ml_dtypes.float8_e4m3fn)
                        # idx stream: [16, cap/16].T tiled to 128 partitions
                        wr = idxs.reshape(cap // 16, 16).T
                        self.eidx[c, :, o16p:o16p + cap // 16] = np.tile(wr, (8, 1))
                        o16p += cap // 16
                        co += cap // P


def build_nc(plan: Plan, gammas):
    nl, nsh, ntot = plan.nl, plan.nsh, plan.ntot
    nhid, ncls, nfeat = plan.nhid, plan.ncls, plan.nfeat
    nb, npair = plan.nb, plan.npair
    nh2 = nhid // P
    nf2 = nfeat // P

    # per-layer scale bookkeeping (host floats)
    fs, xnscale = [], []
    for l in range(nl):
        g = float(gammas[l])
        f = max(abs(g), 1e-4)
        fs.append(f)
        xnscale.append(g / f)

    nc = bacc.Bacc("TRN2", target_bir_lowering=False, debug=False,
                   enable_asserts=True, num_devices=CORES, num_swdge_queues=NQ,
                   dynamic_dma_scratch_size=DMA_SCRATCH)

    xt = nc.dram_tensor("xt", [nfeat, nsh], BF16, kind="ExternalInput").ap()
    wi = nc.dram_tensor("wi", [nfeat, nhid], BF16, kind="ExternalInput").ap()
    birow = nc.dram_tensor("birow", [P, nhid], FP32, kind="ExternalInput").ap()
    vls = nc.dram_tensor("vls", [nl + 1, nhid, ncls], BF16,
                         kind="ExternalInput").ap()
    bs = nc.dram_tensor("bs", [ncls, 1], FP32, kind="ExternalInput").ap()
    eidx = nc.dram_tensor("eidx", [P, plan.tot16], I16, kind="ExternalInput").ap()
    segwt = nc.dram_tensor("segwt", [P, plan.totch, P], F8,
                           kind="ExternalInput").ap()
    gcnt = nc.dram_tensor("gcnt", [1, plan.ncalls], I32,
                          kind="ExternalInput").ap()
    out = nc.dram_tensor("out", [nsh, ncls], FP32, kind="ExternalOutput").ap()

    with tile.TileContext(nc) as tc:
        with tc.tile_pool(name="c1", bufs=1) as c1, \
             tc.tile_pool(name="stream", bufs=3) as st, \
             tc.tile_pool(name="gpool", bufs=3) as gp, \
             tc.tile_pool(name="swpool", bufs=3) as swp, \
             tc.tile_pool(name="ps", bufs=2, space="PSUM") as ps, \
             tc.tile_pool(name="pst", bufs=2, space="PSUM") as pst, \
             tc.tile_pool(name="ps2", bufs=2, space="PSUM") as ps2, \
             tc.tile_pool(name="dram", bufs=1, space="DRAM") as dp:

            # ---------- constants ----------
            iota_i = c1.tile([P, P], I32)
            nc.gpsimd.iota(iota_i[:], pattern=[[1, P]], base=0, channel_multiplier=0)
            iotap_i = c1.tile([P, 1], I32)
            nc.gpsimd.iota(iotap_i[:], pattern=[[1, 1]], base=0, channel_multiplier=1)
            iota_f = c1.tile([P, P], FP32)
            nc.vector.tensor_copy(out=iota_f[:], in_=iota_i[:])
            iotap_f = c1.tile([P, 1], FP32)
            nc.vector.tensor_copy(out=iotap_f[:], in_=iotap_i[:])
            ident = c1.tile([P, P], FP32)
            nc.vector.tensor_tensor(out=ident[:], in0=iota_f[:],
                                    in1=iotap_f[:].to_broadcast([P, P]),
                                    op=ALU.is_equal)
            ident_bf = c1.tile([P, P], BF16)
            nc.vector.tensor_copy(out=ident_bf[:], in_=ident[:])

            wi_sb = c1.tile([P, nf2, nhid], BF16)
            for kc in range(nf2):
                nc.sync.dma_start(out=wi_sb[:, kc, :], in_=wi[kc * P:(kc + 1) * P, :])
            birow_sb = c1.tile([P, nhid], FP32)
            nc.sync.dma_start(out=birow_sb[:], in_=birow[:])
            vl_sb = c1.tile([P, nl + 1, nh2, ncls], BF16)
            for l in range(nl + 1):
                for h in range(nh2):
                    nc.sync.dma_start(out=vl_sb[:, l, h, :],
                                      in_=vls[l, h * P:(h + 1) * P, :])
            bs_sb = c1.tile([ncls, 1], FP32)
            nc.sync.dma_start(out=bs_sb[:], in_=bs[:])

            Xc = c1.tile([P, nb, nhid], BF16, name="Xc")
            outaccT = c1.tile([ncls, nsh], FP32, name="outaccT")

            cnt_t = c1.tile([1, plan.ncalls], I32)
            nc.sync.dma_start(out=cnt_t[:], in_=gcnt[:])
            maxcp = max(len(c) for c in plan.pcalls)
            regsets = [[nc.alloc_register(mybir.EngineType.Pool,
                                          name=f"gcnt{s}_{i}")
                        for i in range(maxcp)] for s in range(2)]

            Ms = [dp.tile([ntot, nhid], F8, addr_space="Shared", name=f"M{l}")
                  for l in range(nl)]

            # ---------- init: H = X @ Wi + birow ----------
            agin = dp.tile([nsh, nhid], F8, name="agin0", bufs=2, tag="agin")
            for b in range(nb):
                cols = slice(b * P, (b + 1) * P)
                xt_sb = st.tile([P, nf2, P], BF16, tag="xt")
                for kc in range(nf2):
                    nc.sync.dma_start(out=xt_sb[:, kc, :],
                                      in_=xt[kc * P:(kc + 1) * P, cols])
                psumH = ps.tile([P, nhid], FP32, tag="ax")
                for kc in range(nf2):
                    nc.tensor.matmul(psumH[:], lhsT=xt_sb[:, kc, :],
                                     rhs=wi_sb[:, kc, :],
                                     start=(kc == 0), stop=(kc == nf2 - 1))
                nc.vector.tensor_add(out=Xc[:, b, :], in0=psumH[:], in1=birow_sb[:])
                h8 = st.tile([P, nhid], F8, tag="x8")
                nc.scalar.activation(out=h8[:], in_=Xc[:, b, :], func=AF.Copy)
                nc.scalar.dma_start(out=agin[cols, :], in_=h8[:])
                # dense init: outaccT = Ws^T @ H^T
                ht = st.tile([P, nh2, P], BF16, tag="axt")
                for h in range(nh2):
                    tps = pst.tile([P, P], BF16, tag="tr")
                    nc.tensor.transpose(out=tps[:], in_=Xc[:, b, h * P:(h + 1) * P],
                                        identity=ident_bf[:])
                    nc.scalar.activation(out=ht[:, h, :], in_=tps[:], func=AF.Copy)
                psum2 = ps2.tile([ncls, P], FP32, tag="mm2")
                for h in range(nh2):
                    nc.tensor.matmul(psum2[:], lhsT=vl_sb[:, 0, h, :],
                                     rhs=ht[:, h, :],
                                     start=(h == 0), stop=(h == nh2 - 1))
                nc.vector.tensor_copy(out=outaccT[:, cols], in_=psum2[:])
            nc.gpsimd.collective_compute(
                "AllGather", ALU.bypass,
                replica_groups=[list(range(CORES))],
                ins=[agin[:]], outs=[Ms[0][:]],
            )

            # ---------- layers ----------
            qrot = 0
            for l in range(nl):
                Mcur = Ms[l]
                last = (l == nl - 1)
                if not last:
                    agin = dp.tile([nsh, nhid], F8, name=f"agin{l+1}",
                                   bufs=2, tag="agin")

                for p in range(npair):
                    ch = int(plan.chp[p])
                    oc = int(plan.ocp[p])
                    calls = plan.pcalls[p]
                    o16p = calls[0][3] if calls else 0
                    ncols16 = sum(ni // 16 for (_, _, ni, _) in calls)

                    idx_t = st.tile([P, max(ncols16, 16)], I16, tag="idx")
                    nc.sync.dma_start(out=idx_t[:, :ncols16],
                                      in_=eidx[:, o16p:o16p + ncols16])

                    G = gp.tile([P, plan.chmax, nhid], F8, tag="g")
                    if l * npair + p < 3:
                        # pool buffers start as SBUF garbage (can be fp8 NaN);
                        # skipped padding descriptors leave slots unwritten, so
                        # zero both buffers once (stale data later is finite).
                        nc.vector.memset(G[:], 0.0)
                    ncp = len(calls)
                    rs = regsets[p % 2]
                    if ncp:
                        pb = int(plan.pbase[p])
                        nc.gpsimd.reg_load(rs[:ncp],
                                           cnt_t[0:1, pb:pb + ncp])
                    for ci, (k, co, ni, o16) in enumerate(calls):
                        lo = k * BUCKET
                        hi = min(lo + BUCKET, ntot)
                        nc.gpsimd.dma_gather(
                            out_ap=G[:, co:co + ni // P, :],
                            in_ap=Mcur[lo:hi, :],
                            idxs_ap=idx_t[:, (o16 - o16p):(o16 - o16p) + ni // 16],
                            num_idxs=ni, num_idxs_reg=rs[ci],
                            elem_size=nhid, queue_num=qrot % NQ,
                        )
                        qrot += 1

                    # segw streamed from DRAM (host-precomputed, static)
                    segw = swp.tile([P, max(ch, KB), P], F8, tag="segw")
                    nc.sync.dma_start(out=segw[:, :ch, :],
                                      in_=segwt[:, oc:oc + ch, :])

                    for h in range(2):
                        b = 2 * p + h
                        cols = slice(b * P, (b + 1) * P)
                        runs = plan.mm[p][h]
                        # DR pairs + orphan singles over chunk runs
                        mms = []
                        for (c0, n) in runs:
                            for j in range(0, n - 1, 2):
                                mms.append((c0 + j, 2))
                            if n % 2:
                                mms.append((c0 + n - 1, 1))
                        psum = ps.tile([P, nhid], FP32, tag="ax")
                        nmm = len(mms)
                        for i, (c0, w) in enumerate(mms):
                            if w == 2:
                                nc.tensor.matmul(
                                    psum[:], lhsT=segw[:, c0:c0 + 2, :],
                                    rhs=G[:, c0:c0 + 2, :],
                                    start=(i == 0), stop=(i == nmm - 1),
                                    perf_mode=PM.DoubleRow)
                            else:
                                nc.tensor.matmul(
                                    psum[:], lhsT=segw[:, c0, :],
                                    rhs=G[:, c0, :],
                                    start=(i == 0), stop=(i == nmm - 1))
                        ax = st.tile([P, nhid], BF16, tag="axs")
                        nc.scalar.activation(out=ax[:], in_=psum[:], func=AF.Copy,
                                             scale=1.0 / 16.0)
                        # dense: outaccT += Vl^T @ AX^T
                        axt = st.tile([P, nh2, P], BF16, tag="axt")
                        for hh in range(nh2):
                            tps = pst.tile([P, P], BF16, tag="tr")
                            nc.tensor.transpose(out=tps[:],
                                                in_=ax[:, hh * P:(hh + 1) * P],
                                                identity=ident_bf[:])
                            nc.scalar.activation(out=axt[:, hh, :], in_=tps[:],
                                                 func=AF.Copy)
                        psum2 = ps2.tile([ncls, P], FP32, tag="mm2")
                        for hh in range(nh2):
                            nc.tensor.matmul(psum2[:], lhsT=vl_sb[:, l + 1, hh, :],
                                             rhs=axt[:, hh, :],
                                             start=(hh == 0), stop=(hh == nh2 - 1))
                        nc.vector.tensor_add(out=outaccT[:, cols],
                                             in0=outaccT[:, cols], in1=psum2[:])
                        if not last:
                            xn = st.tile([P, nhid], BF16, tag="xn")
                            nc.vector.tensor_sub(out=xn[:], in0=Xc[:, b, :], in1=ax[:])
                            nc.scalar.activation(out=Xc[:, b, :], in_=xn[:],
                                                 func=AF.Copy, scale=xnscale[l])
                            x8 = st.tile([P, nhid], F8, tag="x8")
                            nc.scalar.activation(out=x8[:], in_=xn[:],
                                                 func=AF.Copy, scale=xnscale[l])
                            nc.scalar.dma_start(out=agin[cols, :], in_=x8[:])

                if not last:
                    nc.gpsimd.collective_compute(
                        "AllGather", ALU.bypass,
                        replica_groups=[list(range(CORES))],
                        ins=[agin[:]], outs=[Ms[l + 1][:]],
                    )

            # ---------- classifier + log_softmax ----------
            for b in range(nb):
                cols = slice(b * P, (b + 1) * P)
                ot = st.tile([ncls, P], FP32, tag="ot")
                nc.vector.tensor_scalar(out=ot[:], in0=outaccT[:, cols],
                                        scalar1=bs_sb[:], scalar2=None,
                                        op0=ALU.add)
                tps = pst.tile([P, ncls], FP32, tag="trc")
                nc.tensor.transpose(out=tps[:], in_=ot[:],
                                    identity=ident[:ncls, :ncls])
                lg = st.tile([P, ncls], FP32, tag="lg")
                nc.scalar.activation(out=lg[:], in_=tps[:], func=AF.Copy)
                nmx = st.tile([P, 1], FP32, tag="nmx")
                nc.vector.tensor_reduce(out=nmx[:], in_=lg[:],
                                        axis=mybir.AxisListType.X,
                                        op=ALU.max, negate=True)
                ex = st.tile([P, ncls], FP32, tag="ex")
                se = st.tile([P, 1], FP32, tag="se")
                nc.scalar.activation(out=ex[:], in_=lg[:], func=AF.Exp,
                                     bias=nmx[:], scale=1.0, accum_out=se[:])
                lz = st.tile([P, 1], FP32, tag="lz")
                nc.scalar.activation(out=lz[:], in_=se[:], func=AF.Ln)
                ob = st.tile([P, ncls], FP32, tag="ob")
                nc.vector.scalar_tensor_tensor(
                    out=ob[:], in0=lg[:], scalar=nmx[:],
                    in1=lz[:].to_broadcast([P, ncls]),
                    op0=ALU.add, op1=ALU.subtract)
                nc.sync.dma_start(out=out[cols, :], in_=ob[:])

    nc.compile()
    return nc


def run(plan: Plan, X, W_init, b_init, gammas, Ws_l, W_sort, b_sort,
        trace=False):
    nc = build_nc(plan, gammas)

    nl, nsh, nhid, ncls, nfeat = plan.nl, plan.nsh, plan.nhid, plan.ncls, plan.nfeat
    betas = TAU / np.arange(1, nl + 1, dtype=np.float64)
    eye = np.eye(nhid, dtype=np.float64)
    Wso = W_sort.astype(np.float64)

    # vls[0] = Ws (H @ Ws term); vls[1+l] = s_l * Wm_l @ Ws
    vls = np.zeros((nl + 1, nhid, ncls), np.float64)
    vls[0] = Wso
    s = 1.0
    ss = []
    for l in range(nl):
        ss.append(s)
        wm = (1.0 - betas[l]) * eye + betas[l] * Ws_l[l].astype(np.float64)
        vls[1 + l] = s * (wm @ Wso)
        s = s * max(abs(float(gammas[l])), 1e-4)
    vls_bf = vls.astype(ml_dtypes.bfloat16)

    birow = np.tile(b_init.astype(np.float32)[None, :], (P, 1))
    bs2 = b_sort.astype(np.float32).reshape(ncls, 1)

    in_maps = []
    for c in range(CORES):
        xs = np.zeros((nfeat, nsh), np.float32)
        xs[:, :plan.nsh_raw] = X[c * plan.nsh_raw:(c + 1) * plan.nsh_raw].T
        in_maps.append({
            "xt": np.ascontiguousarray(xs.astype(ml_dtypes.bfloat16)),
            "wi": np.ascontiguousarray(W_init.astype(ml_dtypes.bfloat16)),
            "birow": birow, "vls": vls_bf,
            "bs": bs2,
            "eidx": np.ascontiguousarray(plan.eidx[c]),
            "segwt": np.ascontiguousarray(plan.segw[c]),
            "gcnt": np.ascontiguousarray(plan.gcnt[c][None, :]),
        })

    res = bass_utils.run_bass_kernel_spmd(
        nc, in_maps, core_ids=list(range(CORES)),
        trace=trace, trace_cores=[0] if trace else None)

    outs = [res.results[c]["out"][:plan.nsh_raw] for c in range(CORES)]
    return np.concatenate(outs, axis=0), res


def kernel(X, edge_row, edge_col, edge_w, W_init, b_init, gammas, Ws,
           W_sort, b_sort):
    X = np.asarray(X)
    plan = Plan(100000, 512, 256, 64, 8,
                np.asarray(edge_row).astype(np.int64),
                np.asarray(edge_col).astype(np.int64),
                np.asarray(edge_w).astype(np.float32))
    out, _ = run(plan, X, np.asarray(W_init), np.asarray(b_init),
                 np.asarray(gammas), np.asarray(Ws), np.asarray(W_sort),
                 np.asarray(b_sort))
    return out.astype(np.float32)



# revision 50
# speedup vs baseline: 1.0002x; 1.0002x over previous
"""DRC-GCN v2: fp8 gather + DoubleRow staircase on 8 trn2 NeuronCores.

Strategy (node/data parallel, dest-sharded):
  - Destination nodes sharded across 8 cores (12500 -> padded 12544 rows).
  - M_l (the spmm operand) is replicated per core in DRAM as fp8(e4m3),
    value-scaled by 1/s_l (s_l = prod of |gamma|) so it stays in fp8 range.
  - Per layer: gather source rows per edge (256B fp8 descriptors) into
    per-block-pair G tiles; segment-sum via DoubleRow fp8 staircase matmuls
    (lhsT = segw[slot, dest] = 16*w one-hot, built batched on DVE).
  - acc is never materialized: logits accumulate directly through
    host-precomputed Vl = s_l * ((1-b)I + b W_l) @ W_sort  [256, 64].
  - Xc carry kept in SBUF bf16 at 1/s_l scale; Xn cast to fp8 and
    AllGather'd into M_{l+1}.
"""

import math

import numpy as np
import ml_dtypes

import concourse.bass as bass
import concourse.mybir as mybir
import concourse.tile as tile
from concourse import bacc
from concourse import bass_utils

FP32 = mybir.dt.float32
BF16 = mybir.dt.bfloat16
F8 = mybir.dt.float8e4
I16 = mybir.dt.int16
I32 = mybir.dt.int32
AF = mybir.ActivationFunctionType
ALU = mybir.AluOpType
PM = mybir.MatmulPerfMode

P = 128
CORES = 8
TAU = 0.5
BUCKET = 20096          # gather bucket rows (int16-addressable)
NI_MAX = 896            # max idxs per dma_gather call
DMA_SCRATCH = 16384     # SWDGE descriptor ring carveout (bytes/partition)
NQ = 4                  # SWDGE queues
KB = 16                 # segw build batch (chunks per DVE op)


def _round_up(x, m):
    return (x + m - 1) // m * m


class Plan:
    """Static kernel structure + per-core data arrays (block-pair layout)."""

    def __init__(self, n_nodes, nfeat, nhid, ncls, nlayers,
                 edge_row, edge_col, edge_w):
        self.n = n_nodes
        self.nfeat = nfeat
        self.nhid = nhid
        self.ncls = ncls
        self.nl = nlayers
        assert n_nodes % CORES == 0
        self.nsh_raw = n_nodes // CORES
        self.nsh = _round_up(self.nsh_raw, P)
        self.nb = self.nsh // P
        assert self.nb % 2 == 0
        self.npair = self.nb // 2
        self.ntot = self.nsh * CORES
        self.nbuck = (self.ntot + BUCKET - 1) // BUCKET

        own = edge_col // self.nsh_raw
        gsrc = own * self.nsh + (edge_col - own * self.nsh_raw)
        dst_core = edge_row // self.nsh_raw
        ldst = edge_row - dst_core * self.nsh_raw
        blk = ldst // P
        pair = blk // 2
        half = blk % 2
        ld_in_blk = ldst % P
        bucket = gsrc // BUCKET
        lidx = gsrc - bucket * BUCKET

        npair, nbuck = self.npair, self.nbuck
        counts = np.zeros((CORES, npair, nbuck, 2), np.int64)
        np.add.at(counts, (dst_core, pair, bucket, half), 1)
        caps = _round_up(counts.max(axis=0), P)            # [npair, nbuck, 2]
        self.caps = caps

        # chunk layout per pair: bucket-major, [A chunks | B chunks]
        # calls per (pair, bucket): one if niA+niB <= NI_MAX else two
        self.pcalls = []      # per pair: list of (bucket, chunk_off, ni, o16)
        self.mm = []          # per pair: [half] -> list of (c0, n) chunk runs
        self.chp = np.zeros(npair, np.int64)
        o16 = 0

        def _pieces(cap):
            # split cap into balanced multiples of 128, each <= NI_MAX
            out = []
            left = cap
            while left > 0:
                ni = min(left, NI_MAX)
                if left > NI_MAX:
                    ni = min(_round_up(left // 2, P), NI_MAX)
                out.append(ni)
                left -= ni
            return out

        # calls split per (pair, bucket, half) so per-core padding is always
        # trailing within a call; gcnt holds per-core real counts (>=16,
        # 16-aligned, trailing idxs set to -1) so the gather ucode skips
        # padding descriptors.
        self.cmeta = {}       # (pair, bucket, half) -> [(call_gidx, off, ni)]
        gidx = 0
        for p in range(npair):
            calls = []
            runs = [[], []]
            co = 0
            for k in range(nbuck):
                capA = int(caps[p, k, 0])
                capB = int(caps[p, k, 1])
                if capA + capB == 0:
                    continue
                cco = co
                for h, cap in ((0, capA), (1, capB)):
                    off = 0
                    metas = []
                    for ni in _pieces(cap):
                        calls.append((k, cco, ni, o16))
                        metas.append((gidx, off, ni))
                        gidx += 1
                        o16 += ni // 16
                        cco += ni // P
                        off += ni
                    if metas:
                        self.cmeta[(p, k, h)] = metas
                if capA:
                    runs[0].append((co, capA // P))
                if capB:
                    runs[1].append((co + capA // P, capB // P))
                co += (capA + capB) // P
            self.pcalls.append(calls)
            self.mm.append(runs)
            self.chp[p] = co
        self.ocp = np.concatenate([[0], np.cumsum(self.chp)]).astype(np.int64)
        self.totch = int(self.ocp[-1])
        self.chmax = int(self.chp.max())
        self.tot16 = o16
        self.ncalls = gidx
        self.pbase = np.concatenate(
            [[0], np.cumsum([len(c) for c in self.pcalls])]).astype(np.int64)
        self.gcnt = np.zeros((CORES, self.ncalls), np.int32)

        # ---- per-core data arrays ----
        order = np.lexsort((lidx, half, bucket, pair, dst_core))
        self.eidx = np.zeros((CORES, P, self.tot16), np.int16)
        self.segw = np.zeros((CORES, P, self.totch, P), ml_dtypes.float8_e4m3fn)

        s_core = dst_core[order]
        s_pair = pair[order]
        s_buck = bucket[order]
        s_half = half[order]
        s_lidx = lidx[order]
        s_ld = ld_in_blk[order]
        s_w = edge_w[order]

        core_starts = np.searchsorted(s_core, np.arange(CORES + 1))
        nseg = npair * nbuck * 2
        for c in range(CORES):
            a0, a1 = core_starts[c], core_starts[c + 1]
            key = (s_pair[a0:a1] * nbuck + s_buck[a0:a1]) * 2 + s_half[a0:a1]
            seg_starts = np.searchsorted(key, np.arange(nseg + 1)) + a0
            for p in range(npair):
                co = 0
                o16p = self.pcalls[p][0][3] if self.pcalls[p] else 0
                for k in range(self.nbuck):
                    for h in range(2):
                        cap = int(caps[p, k, h])
                        if cap == 0:
                            continue
                        s0 = seg_starts[(p * nbuck + k) * 2 + h]
                        s1 = seg_starts[(p * nbuck + k) * 2 + h + 1]
                        nreal = s1 - s0
                        idxs = np.full(cap, -1, np.int16)
                        idxs[:nreal] = s_lidx[s0:s1]
                        for (gi, off, ni) in self.cmeta.get((p, k, h), []):
                            real = min(max(nreal - off, 0), ni)
                            eff = min(ni, _round_up(max(real, 16), 16))
                            idxs[off + real:off + eff] = 0
                            self.gcnt[c, gi] = eff
                        jr = np.arange(nreal)
                        gco = self.ocp[p] + co
                        self.segw[c, jr % P, gco + jr // P,
                                  s_ld[s0:s1]] = np.clip(
                            16.0 * s_w[s0:s1], 0, 240).astype(
                            ml_dtypes.float8_e4m3fn)
                        # idx stream: [16, cap/16].T tiled to 128 partitions
                        wr = idxs.reshape(cap // 16, 16).T
                        self.eidx[c, :, o16p:o16p + cap // 16] = np.tile(wr, (8, 1))
                        o16p += cap // 16
                        co += cap // P


def build_nc(plan: Plan, gammas):
    nl, nsh, ntot = plan.nl, plan.nsh, plan.ntot
    nhid, ncls, nfeat = plan.nhid, plan.ncls, plan.nfeat
    nb, npair = plan.nb, plan.npair
    nh2 = nhid // P
    nf2 = nfeat // P

    # per-layer scale bookkeeping (host floats)
    fs, xnscale = [], []
    for l in range(nl):
        g = float(gammas[l])
        f = max(abs(g), 1e-4)
        fs.append(f)
        xnscale.append(g / f)

    nc = bacc.Bacc("TRN2", target_bir_lowering=False, debug=False,
                   enable_asserts=True, num_devices=CORES, num_swdge_queues=NQ,
                   dynamic_dma_scratch_size=DMA_SCRATCH)

    xt = nc.dram_tensor("xt", [nfeat, nsh], BF16, kind="ExternalInput").ap()
    wi = nc.dram_tensor("wi", [nfeat, nhid], BF16, kind="ExternalInput").ap()
    birow = nc.dram_tensor("birow", [P, nhid], FP32, kind="ExternalInput").ap()
    vls = nc.dram_tensor("vls", [nl + 1, nhid, ncls], BF16,
                         kind="ExternalInput").ap()
    bs = nc.dram_tensor("bs", [ncls, 1], FP32, kind="ExternalInput").ap()
    eidx = nc.dram_tensor("eidx", [P, plan.tot16], I16, kind="ExternalInput").ap()
    segwt = nc.dram_tensor("segwt", [P, plan.totch, P], F8,
                           kind="ExternalInput").ap()
    gcnt = nc.dram_tensor("gcnt", [1, plan.ncalls], I32,
                          kind="ExternalInput").ap()
    out = nc.dram_tensor("out", [nsh, ncls], FP32, kind="ExternalOutput").ap()

    with tile.TileContext(nc) as tc:
        with tc.tile_pool(name="c1", bufs=1) as c1, \
             tc.tile_pool(name="stream", bufs=3) as st, \
             tc.tile_pool(name="gpool", bufs=2) as gp, \
             tc.tile_pool(name="swpool", bufs=2) as swp, \
             tc.tile_pool(name="ps", bufs=2, space="PSUM") as ps, \
             tc.tile_pool(name="pst", bufs=2, space="PSUM") as pst, \
             tc.tile_pool(name="ps2", bufs=2, space="PSUM") as ps2, \
             tc.tile_pool(name="dram", bufs=1, space="DRAM") as dp:

            # ---------- constants ----------
            iota_i = c1.tile([P, P], I32)
            nc.gpsimd.iota(iota_i[:], pattern=[[1, P]], base=0, channel_multiplier=0)
            iotap_i = c1.tile([P, 1], I32)
            nc.gpsimd.iota(iotap_i[:], pattern=[[1, 1]], base=0, channel_multiplier=1)
            iota_f = c1.tile([P, P], FP32)
            nc.vector.tensor_copy(out=iota_f[:], in_=iota_i[:])
            iotap_f = c1.tile([P, 1], FP32)
            nc.vector.tensor_copy(out=iotap_f[:], in_=iotap_i[:])
            ident = c1.tile([P, P], FP32)
            nc.vector.tensor_tensor(out=ident[:], in0=iota_f[:],
                                    in1=iotap_f[:].to_broadcast([P, P]),
                                    op=ALU.is_equal)
            ident_bf = c1.tile([P, P], BF16)
            nc.vector.tensor_copy(out=ident_bf[:], in_=ident[:])

            wi_sb = c1.tile([P, nf2, nhid], BF16)
            for kc in range(nf2):
                nc.sync.dma_start(out=wi_sb[:, kc, :], in_=wi[kc * P:(kc + 1) * P, :])
            birow_sb = c1.tile([P, nhid], FP32)
            nc.sync.dma_start(out=birow_sb[:], in_=birow[:])
            vl_sb = c1.tile([P, nl + 1, nh2, ncls], BF16)
            for l in range(nl + 1):
                for h in range(nh2):
                    nc.sync.dma_start(out=vl_sb[:, l, h, :],
                                      in_=vls[l, h * P:(h + 1) * P, :])
            bs_sb = c1.tile([ncls, 1], FP32)
            nc.sync.dma_start(out=bs_sb[:], in_=bs[:])

            Xc = c1.tile([P, nb, nhid], BF16, name="Xc")
            outaccT = c1.tile([ncls, nsh], FP32, name="outaccT")

            cnt_t = c1.tile([1, plan.ncalls], I32)
            nc.sync.dma_start(out=cnt_t[:], in_=gcnt[:])
            maxcp = max(len(c) for c in plan.pcalls)
            regsets = [[nc.alloc_register(mybir.EngineType.Pool,
                                          name=f"gcnt{s}_{i}")
                        for i in range(maxcp)] for s in range(2)]

            Ms = [dp.tile([ntot, nhid], F8, addr_space="Shared", name=f"M{l}")
                  for l in range(nl)]

            # ---------- init: H = X @ Wi + birow ----------
            agin = dp.tile([nsh, nhid], F8, name="agin0", bufs=2, tag="agin")
            for b in range(nb):
                cols = slice(b * P, (b + 1) * P)
                xt_sb = st.tile([P, nf2, P], BF16, tag="xt")
                for kc in range(nf2):
                    nc.sync.dma_start(out=xt_sb[:, kc, :],
                                      in_=xt[kc * P:(kc + 1) * P, cols])
                psumH = ps.tile([P, nhid], FP32, tag="ax")
                for kc in range(nf2):
                    nc.tensor.matmul(psumH[:], lhsT=xt_sb[:, kc, :],
                                     rhs=wi_sb[:, kc, :],
                                     start=(kc == 0), stop=(kc == nf2 - 1))
                nc.vector.tensor_add(out=Xc[:, b, :], in0=psumH[:], in1=birow_sb[:])
                h8 = st.tile([P, nhid], F8, tag="x8")
                nc.scalar.activation(out=h8[:], in_=Xc[:, b, :], func=AF.Copy)
                nc.scalar.dma_start(out=agin[cols, :], in_=h8[:])
                # dense init: outaccT = Ws^T @ H^T
                ht = st.tile([P, nh2, P], BF16, tag="axt")
                for h in range(nh2):
                    tps = pst.tile([P, P], BF16, tag="tr")
                    nc.tensor.transpose(out=tps[:], in_=Xc[:, b, h * P:(h + 1) * P],
                                        identity=ident_bf[:])
                    nc.scalar.activation(out=ht[:, h, :], in_=tps[:], func=AF.Copy)
                psum2 = ps2.tile([ncls, P], FP32, tag="mm2")
                for h in range(nh2):
                    nc.tensor.matmul(psum2[:], lhsT=vl_sb[:, 0, h, :],
                                     rhs=ht[:, h, :],
                                     start=(h == 0), stop=(h == nh2 - 1))
                nc.vector.tensor_copy(out=outaccT[:, cols], in_=psum2[:])
            nc.gpsimd.collective_compute(
                "AllGather", ALU.bypass,
                replica_groups=[list(range(CORES))],
                ins=[agin[:]], outs=[Ms[0][:]],
            )

            # ---------- layers ----------
            qrot = 0
            for l in range(nl):
                Mcur = Ms[l]
                last = (l == nl - 1)
                if not last:
                    agin = dp.tile([nsh, nhid], F8, name=f"agin{l+1}",
                                   bufs=2, tag="agin")

                for p in range(npair):
                    ch = int(plan.chp[p])
                    oc = int(plan.ocp[p])
                    calls = plan.pcalls[p]
                    o16p = calls[0][3] if calls else 0
                    ncols16 = sum(ni // 16 for (_, _, ni, _) in calls)

                    idx_t = st.tile([P, max(ncols16, 16)], I16, tag="idx")
                    nc.sync.dma_start(out=idx_t[:, :ncols16],
                                      in_=eidx[:, o16p:o16p + ncols16])

                    G = gp.tile([P, plan.chmax, nhid], F8, tag="g")
                    if l * npair + p < 2:
                        # pool buffers start as SBUF garbage (can be fp8 NaN);
                        # skipped padding descriptors leave slots unwritten, so
                        # zero both buffers once (stale data later is finite).
                        nc.vector.memset(G[:], 0.0)
                    ncp = len(calls)
                    rs = regsets[p % 2]
                    if ncp:
                        pb = int(plan.pbase[p])
                        nc.gpsimd.reg_load(rs[:ncp],
                                           cnt_t[0:1, pb:pb + ncp])
                    for ci, (k, co, ni, o16) in enumerate(calls):
                        lo = k * BUCKET
                        hi = min(lo + BUCKET, ntot)
                        nc.gpsimd.dma_gather(
                            out_ap=G[:, co:co + ni // P, :],
                            in_ap=Mcur[lo:hi, :],
                            idxs_ap=idx_t[:, (o16 - o16p):(o16 - o16p) + ni // 16],
                            num_idxs=ni, num_idxs_reg=rs[ci],
                            elem_size=nhid, queue_num=qrot % NQ,
                        )
                        qrot += 1

                    # segw streamed from DRAM (host-precomputed, static)
                    segw = swp.tile([P, max(ch, KB), P], F8, tag="segw")
                    nc.sync.dma_start(out=segw[:, :ch, :],
                                      in_=segwt[:, oc:oc + ch, :])

                    for h in range(2):
                        b = 2 * p + h
                        cols = slice(b * P, (b + 1) * P)
                        runs = plan.mm[p][h]
                        # DR pairs + orphan singles over chunk runs
                        mms = []
                        for (c0, n) in runs:
                            for j in range(0, n - 1, 2):
                                mms.append((c0 + j, 2))
                            if n % 2:
                                mms.append((c0 + n - 1, 1))
                        psum = ps.tile([P, nhid], FP32, tag="ax")
                        nmm = len(mms)
                        for i, (c0, w) in enumerate(mms):
                            if w == 2:
                                nc.tensor.matmul(
                                    psum[:], lhsT=segw[:, c0:c0 + 2, :],
                                    rhs=G[:, c0:c0 + 2, :],
                                    start=(i == 0), stop=(i == nmm - 1),
                                    perf_mode=PM.DoubleRow)
                            else:
                                nc.tensor.matmul(
                                    psum[:], lhsT=segw[:, c0, :],
                                    rhs=G[:, c0, :],
                                    start=(i == 0), stop=(i == nmm - 1))
                        ax = st.tile([P, nhid], BF16, tag="axs")
                        nc.scalar.activation(out=ax[:], in_=psum[:], func=AF.Copy,
                                             scale=1.0 / 16.0)
                        # dense: outaccT += Vl^T @ AX^T
                        axt = st.tile([P, nh2, P], BF16, tag="axt")
                        for hh in range(nh2):
                            tps = pst.tile([P, P], BF16, tag="tr")
                            nc.tensor.transpose(out=tps[:],
                                                in_=ax[:, hh * P:(hh + 1) * P],
                                                identity=ident_bf[:])
                            nc.scalar.activation(out=axt[:, hh, :], in_=tps[:],
                                                 func=AF.Copy)
                        psum2 = ps2.tile([ncls, P], FP32, tag="mm2")
                        for hh in range(nh2):
                            nc.tensor.matmul(psum2[:], lhsT=vl_sb[:, l + 1, hh, :],
                                             rhs=axt[:, hh, :],
                                             start=(hh == 0), stop=(hh == nh2 - 1))
                        nc.vector.tensor_add(out=outaccT[:, cols],
                                             in0=outaccT[:, cols], in1=psum2[:])
                        if not last:
                            xn = st.tile([P, nhid], BF16, tag="xn")
                            nc.vector.tensor_sub(out=xn[:], in0=Xc[:, b, :], in1=ax[:])
                            nc.scalar.activation(out=Xc[:, b, :], in_=xn[:],
                                                 func=AF.Copy, scale=xnscale[l])
                            x8 = st.tile([P, nhid], F8, tag="x8")
                            nc.scalar.activation(out=x8[:], in_=xn[:],
                                                 func=AF.Copy, scale=xnscale[l])
                            nc.scalar.dma_start(out=agin[cols, :], in_=x8[:])

                if not last:
                    nc.gpsimd.collective_compute(
                        "AllGather", ALU.bypass,
                        replica_groups=[list(range(CORES))],
                        ins=[agin[:]], outs=[Ms[l + 1][:]],
                    )

            # ---------- classifier + log_softmax ----------
            for b in range(nb):
                cols = slice(b * P, (b + 1) * P)
                ot = st.tile([ncls, P], FP32, tag="ot")
                nc.vector.tensor_scalar(out=ot[:], in0=outaccT[:, cols],
                                        scalar1=bs_sb[:], scalar2=None,
                                        op0=ALU.add)
                tps = pst.tile([P, ncls], FP32, tag="trc")
                nc.tensor.transpose(out=tps[:], in_=ot[:],
                                    identity=ident[:ncls, :ncls])
                lg = st.tile([P, ncls], FP32, tag="lg")
                nc.scalar.activation(out=lg[:], in_=tps[:], func=AF.Copy)
                nmx = st.tile([P, 1], FP32, tag="nmx")
                nc.vector.tensor_reduce(out=nmx[:], in_=lg[:],
                                        axis=mybir.AxisListType.X,
                                        op=ALU.max, negate=True)
                ex = st.tile([P, ncls], FP32, tag="ex")
                se = st.tile([P, 1], FP32, tag="se")
                nc.scalar.activation(out=ex[:], in_=lg[:], func=AF.Exp,
                                     bias=nmx[:], scale=1.0, accum_out=se[:])
                lz = st.tile([P, 1], FP32, tag="lz")
                nc.scalar.activation(out=lz[:], in_=se[:], func=AF.Ln)
                ob = st.tile([P, ncls], FP32, tag="ob")
                nc.vector.scalar_tensor_tensor(
                    out=ob[:], in0=lg[:], scalar=nmx[:],
                    in1=lz[:].to_broadcast([P, ncls]),
                    op0=ALU.add, op1=ALU.subtract)
                nc.sync.dma_start(out=out[cols, :], in_=ob[:])

    nc.compile()
    return nc


def run(plan: Plan, X, W_init, b_init, gammas, Ws_l, W_sort, b_sort,
        trace=False):
    nc = build_nc(plan, gammas)

    nl, nsh, nhid, ncls, nfeat = plan.nl, plan.nsh, plan.nhid, plan.ncls, plan.nfeat
    betas = TAU / np.arange(1, nl + 1, dtype=np.float64)
    eye = np.eye(nhid, dtype=np.float64)
    Wso = W_sort.astype(np.float64)

    # vls[0] = Ws (H @ Ws term); vls[1+l] = s_l * Wm_l @ Ws
    vls = np.zeros((nl + 1, nhid, ncls), np.float64)
    vls[0] = Wso
    s = 1.0
    ss = []
    for l in range(nl):
        ss.append(s)
        wm = (1.0 - betas[l]) * eye + betas[l] * Ws_l[l].astype(np.float64)
        vls[1 + l] = s * (wm @ Wso)
        s = s * max(abs(float(gammas[l])), 1e-4)
    vls_bf = vls.astype(ml_dtypes.bfloat16)

    birow = np.tile(b_init.astype(np.float32)[None, :], (P, 1))
    bs2 = b_sort.astype(np.float32).reshape(ncls, 1)

    in_maps = []
    for c in range(CORES):
        xs = np.zeros((nfeat, nsh), np.float32)
        xs[:, :plan.nsh_raw] = X[c * plan.nsh_raw:(c + 1) * plan.nsh_raw].T
        in_maps.append({
            "xt": np.ascontiguousarray(xs.astype(ml_dtypes.bfloat16)),
            "wi": np.ascontiguousarray(W_init.astype(ml_dtypes.bfloat16)),
            "birow": birow, "vls": vls_bf,
            "bs": bs2,
            "eidx": np.ascontiguousarray(plan.eidx[c]),
            "segwt": np.ascontiguousarray(plan.segw[c]),
            "gcnt": np.ascontiguousarray(plan.gcnt[c][None, :]),
        })

    res = bass_utils.run_bass_kernel_spmd(
        nc, in_maps, core_ids=list(range(CORES)),
        trace=trace, trace_cores=[0] if trace else None)

    outs = [res.results[c]["out"][:plan.nsh_raw] for c in range(CORES)]
    return np.concatenate(outs, axis=0), res


def kernel(X, edge_row, edge_col, edge_w, W_init, b_init, gammas, Ws,
           W_sort, b_sort):
    X = np.asarray(X)
    plan = Plan(100000, 512, 256, 64, 8,
                np.asarray(edge_row).astype(np.int64),
                np.asarray(edge_col).astype(np.int64),
                np.asarray(edge_w).astype(np.float32))
    out, _ = run(plan, X, np.asarray(W_init), np.asarray(b_init),
                 np.asarray(gammas), np.asarray(Ws), np.asarray(W_sort),
                 np.asarray(b_sort))
    return out.astype(np.float32)

